# revision 42
# baseline (speedup 1.0000x reference)
"""CorrelationFusion Trainium2 kernel.

Per-clip math (T=8 frames, G=4 groups, 3x3 correlation window):
  corr[g, tt*9+ij, p] = sum_cp x[tt, g*64+cp, p] * xpad[tt+1, g*64+cp, p+d(ij)]
  wx[g, o*8+t, p]     = sum_i conv_w[g, o*8+t, i]*corr[g, i, p] + conv_b[g, o*8+t]
  out[o, g*64+cp, p]  = sum_t wx'[g, o*8+t, p] * x[t, cp*4+g, p]
  (wx' = wx + 1 on the t==o rows -- the residual folded into the conv bias)

Mapping:
  - per-pixel products on VectorE in bf16 (2x mode), channels on partitions
  - partition reductions via TensorE matmuls, 4-way COLUMN-TILED
    (tile_position col groups) so 4 small-M matmuls run concurrently
  - the 1x1 grouped conv is a block-diagonal matmul over the 63 corr rows
    (k-rows permuted to match the col-tiled corr psum layout)
  - padded frame tiles filled directly by strided DMA (no staging copies);
    edge replication pads on ScalarE
  - wx rows replicated into the (cp16, t8)-interleaved layout via a
    broadcast-read DMA from a DRAM bounce buffer
  - data-parallel over the 8 clips: one clip per NeuronCore
"""

import numpy as np
import ml_dtypes

T = 8
TO = 8
G = 4
CPG = 64
C = 256
H = 56
W = 56
PIX = H * W
NCORES = 8
PH = 58   # padded tile rows
PW = 60   # padded tile cols (extra pad for 4B alignment of bf16 rows)
NCH = 7   # pixel chunks per image
CHW = 8   # rows per chunk
CHN = CHW * W  # 448 pixels per chunk
_CACHE = {}


def _corr_row(i, gh):
    """psum/partition row of corr index i (0..62) for channel-half gh."""
    return 32 * (i % 4) + 2 * (i // 4) + gh


def _build_module(loop_k=1):
    import contextlib

    import concourse.bass as bass
    import concourse.bacc as bacc
    import concourse.mybir as mybir
    import concourse.tile as tile

    fp32 = mybir.dt.float32
    bf16 = mybir.dt.bfloat16

    nc = bacc.Bacc(name="corrfusion")
    xin = nc.dram_tensor("xin", [T, C, H, W], bf16, kind="ExternalInput")
    wf2 = nc.dram_tensor("wf2", [128, 2, 128], fp32, kind="ExternalInput")
    bm2 = nc.dram_tensor("bm2", [128, 96], bf16, kind="ExternalInput")
    tones = nc.dram_tensor("tones", [128, 4, 16], bf16, kind="ExternalInput")
    bvec = nc.dram_tensor("bvec", [128, 2], fp32, kind="ExternalInput")
    out = nc.dram_tensor("out", [TO, C, H, W], fp32, kind="ExternalOutput")

    xin_base = xin[:, :, :, :]                                 # base AP for manual APs
    out_r = out.rearrange("o (g cpc k) h w -> o g cpc k (h w)", g=4, cpc=4, k=16)

    with tile.TileContext(nc) as tc:
        with tc.tile_pool(name="consts", bufs=1) as consts, \
             tc.tile_pool(name="corrbuf", bufs=1) as corrbuf, \
             tc.tile_pool(name="xt", bufs=2) as xtp, \
             tc.tile_pool(name="wxdp", bufs=1, space="DRAM") as wxdp, \
             tc.tile_pool(name="psum", bufs=1, space="PSUM") as psum:

            wf_sb = consts.tile([128, 2, 128], fp32)
            nc.scalar.dma_start(out=wf_sb, in_=wf2[:, :, :])
            bm_sb = consts.tile([128, 96], bf16)
            nc.scalar.dma_start(out=bm_sb, in_=bm2[:, :])
            to_sb = consts.tile([128, 4, 16], bf16)
            nc.scalar.dma_start(out=to_sb, in_=tones[:, :, :])
            bv_sb = consts.tile([128, 2], fp32)
            nc.scalar.dma_start(out=bv_sb, in_=bvec[:, :])

            corr_sb = [
                corrbuf.tile([128, PIX], fp32, tag=f"corr{i}", name=f"corr{i}")
                for i in range(2)
            ]
            wx_sb = [
                corrbuf.tile([128, PIX], bf16, tag=f"wx{i}", name=f"wx{i}")
                for i in range(2)
            ]
            loop_cm = (
                tc.For_i(0, loop_k, 1) if loop_k > 1 else contextlib.nullcontext()
            )
            with loop_cm:
                _build_body(nc, tc, tile, bass, mybir, fp32, bf16, xin, xin_base,
                            out_r, wf_sb, bm_sb, to_sb, bv_sb, corr_sb, wx_sb,
                            xtp, wxdp, psum)
    nc.compile()
    return nc


def _build_body(nc, tc, tile, bass, mybir, fp32, bf16, xin, xin_base, out_r,
                wf_sb, bm_sb, to_sb, bv_sb, corr_sb, wx_sb, xtp, wxdp, psum):
    # rows 126/127 (unused by the col-tiled corr layout) must read as zeros
    # in the conv matmul; memset the whole 32-row group (engines need a
    # 32-aligned partition base), the drains overwrite rows 96..125
    for i in range(2):
        nc.vector.memset(corr_sb[i][96:128, :], 0.0)

    wxd = [None, None]

    frames_pools = {}  # set by the caller: frames, stage, prods
    ptiles = {0: {}, 1: {}}
    stiles = {0: {}, 1: {}}

    def load_frame(ct, t, warmup=False):
        frames, stage, prods = frames_pools["pools"]
        ptile, stile = ptiles[ct], stiles[ct]
        P = frames.tile([128, PH, PW], bf16, tag=f"P{t % 4}", name=f"P{ct}_{t}")
        # contiguous DMA into a staging tile; strided center copies on
        # compute engines (strided DMA writes are packet-inefficient)
        stg = stage.tile([128, PIX], bf16, tag="fstage", name="stg")
        nc.sync.dma_start(out=stg, in_=xin[t, ct * 128:(ct + 1) * 128, :, :])
        stg3 = stg.rearrange("p (h w) -> p h w", h=H)

        # during warmup VectorE is idle -- let it carry the S-tile work
        def scopy(o_, i_):
            if warmup:
                nc.vector.tensor_copy(o_, i_)
            else:
                nc.scalar.copy(o_, i_)

        if warmup:
            nc.vector.tensor_copy(P[:, 1:57, 2:58], stg3)
        else:
            nc.scalar.copy(P[:, 1:57, 2:58], stg3)
        ptile[t] = P
        if t > 0:
            # frame 0 is only ever the 'a' operand: its S tile and the
            # P row pads (which serve di-shifted 'b' reads) are never read
            nc.scalar.copy(P[:, 0:1, 2:58], P[:, 1:2, 2:58])
            nc.scalar.copy(P[:, 57:58, 2:58], P[:, 56:57, 2:58])
            S = frames.tile([128, PH, PW], bf16, tag=f"S{t % 4}", name=f"S{ct}_{t}")
            scopy(S[:, 1:57, 1:57], stg3)
            # S is read at cols 0:56 / 2:58 (dj=-1/+1): col pads 0 and 57
            scopy(S[:, 1:57, 0:1], S[:, 1:57, 1:2])
            scopy(S[:, 1:57, 57:58], S[:, 1:57, 56:57])
            scopy(S[:, 0:1, 0:58], S[:, 1:2, 0:58])
            scopy(S[:, 57:58, 0:58], S[:, 56:57, 0:58])
            stile[t] = S

    def corr_phase(ct):
        if True:
            frames, stage, prods = frames_pools["pools"]
            cps = [
                psum.tile([128, CHN], fp32, tag=f"b{c}", name=f"cps{ct}_{c}")
                for c in range(NCH)
            ]
            ptile = ptiles[ct]
            stile = stiles[ct]

            if 0 not in ptile:
                load_frame(ct, 0, warmup=(ct == 0))
            if 1 not in ptile:
                load_frame(ct, 1, warmup=(ct == 0))
            for tt in range(T - 1):
                if tt + 2 < T:
                    load_frame(ct, tt + 2)
                if ct == 0 and tt == T - 2:
                    # preload next half's first frames while this half's
                    # tail products still run (keeps the scalar queue from
                    # serializing drains ahead of them)
                    load_frame(1, 0)
                    load_frame(1, 1)
                a = ptile[tt][:, 1:57, 2:58]
                for ij in range(9):
                    di = ij // 3 - 1
                    dj = ij % 3 - 1
                    r = tt * 9 + ij
                    if dj == 0:
                        b = ptile[tt + 1][:, 1 + di:57 + di, 2:58]
                    elif dj == 1:
                        b = stile[tt + 1][:, 1 + di:57 + di, 2:58]
                    else:
                        b = stile[tt + 1][:, 1 + di:57 + di, 0:56]
                    pr = prods.tile([128, PIX], bf16, tag="prod", name="pr")
                    pr3 = pr.rearrange("p (h w) -> p h w", h=H)
                    nc.vector.tensor_mul(pr3, a, b)
                    # col-tiled reduction: product r -> col group r%4,
                    # rows 2*(r//4)+gh; lhsT = 32-col slice of the shifted
                    # ones matrix
                    cg = r % 4
                    q = r // 4
                    lhsT = bm_sb[:, 62 - 2 * q:94 - 2 * q]
                    last = (q == 15) or (cg == 3 and q == 14)
                    for c in range(NCH):
                        nc.tensor.matmul(
                            cps[c][32 * cg:32 * cg + 32, :],
                            lhsT,
                            pr[:, c * CHN:(c + 1) * CHN],
                            start=(q == 0),
                            stop=last,
                            tile_position=(0, 32 * cg),
                        )
            for c in range(NCH):
                nc.scalar.copy(
                    corr_sb[ct][0:126, c * CHN:(c + 1) * CHN],
                    cps[c][0:126, :],
                )

    def conv_phase(gp):
        # grouped 1x1 conv (+bias +residual); bounce wx to DRAM so the
        # per-(o,g) replication is one broadcast-read DMA
        for c in range(NCH):
            wpp = psum.tile([128, CHN], fp32, tag="wp", name=f"wpp{gp}_{c}")
            nc.tensor.matmul(
                wpp,
                wf_sb[:, gp, :],
                corr_sb[gp][:, c * CHN:(c + 1) * CHN],
                start=True,
                stop=True,
            )
            nc.scalar.activation(
                wx_sb[gp][:, c * CHN:(c + 1) * CHN],
                wpp,
                mybir.ActivationFunctionType.Identity,
                bias=bv_sb[:, gp:gp + 1],
                scale=1.0,
            )
        # trigger on the scalar queue: its dep (the wx activations) sits
        # right before it there, so it never blocks a queue head
        wd = wxdp.tile([128, PIX], bf16, tag=f"wxd{gp}", name=f"wxd{gp}")
        nc.scalar.dma_start(out=wd, in_=wx_sb[gp])
        wxd[gp] = wd

    xt_tiles = {}
    wrep_tiles = {}
    wrepp_box = {}

    def load_xt(g):
        if g in xt_tiles:
            return xt_tiles[g]
        xt = xtp.tile([128, 4, PIX], bf16, tag="xt", name=f"xt{g}")
        for cpc in range(4):
            # partition = (cpk, t): channel c = cpc*64 + cpk*4 + g
            src = bass.AP(
                tensor=xin_base.tensor,
                offset=(cpc * 64 + g) * PIX,
                ap=[[4 * PIX, 16], [C * PIX, T], [1, PIX]],
            )
            nc.sync.dma_start(out=xt[:, cpc, :], in_=src)
        xt_tiles[g] = xt
        return xt

    def make_wrep(g, o):
        if (g, o) in wrep_tiles:
            return wrep_tiles.pop((g, o))
        rowbase = (g % 2) * 64 + o * 8
        wrep = wrepp_box["pool"].tile([128, PIX], bf16, tag="wrep", name="wrep")
        wsrc = bass.AP(
            tensor=wxd[g // 2].tensor,
            offset=wxd[g // 2].offset + rowbase * PIX,
            ap=[[0, 16], [PIX, 8], [1, PIX]],
        )
        nc.sync.dma_start(out=wrep, in_=wsrc)
        return wrep

    def phase2(g, pr2p, xobp):
        xt = load_xt(g)
        for o in range(TO):
            wrep = make_wrep(g, o)
            # for the very last (g, o) compute products chunk-wise so the
            # trailing MM/drain/DMA chain starts ~6us earlier
            fine = (g == 3 and o == TO - 1)
            # cpc-pair products in one DVE op: wrep broadcast over cpc
            pr4 = pr2p.tile([128, 4, PIX], bf16, tag="pr2", bufs=3, name="pr4")
            if not fine:
                wb = wrep[:, :].unsqueeze(1).broadcast_to((128, 2, PIX))
                nc.vector.tensor_mul(pr4[:, 0:2, :], xt[:, 0:2, :], wb)
                nc.vector.tensor_mul(pr4[:, 2:4, :], xt[:, 2:4, :], wb)
            xout = xobp.tile([128, PIX], fp32, tag="xout", name="xout")
            for c in range(NCH):
                cs, ce = c * CHN, (c + 1) * CHN
                if fine:
                    wbc = wrep[:, cs:ce].unsqueeze(1).broadcast_to((128, 4, CHN))
                    nc.vector.tensor_mul(
                        pr4[:, :, cs:ce], xt[:, :, cs:ce], wbc
                    )
                xop = psum.tile([128, CHN], fp32, tag=f"b{c}", name=f"xo{g}_{o}_{c}")
                # 4 col-tiled t-reductions run concurrently
                for cpc in range(4):
                    nc.tensor.matmul(
                        xop[32 * cpc:32 * cpc + 16, :],
                        to_sb[:, cpc, :],
                        pr4[:, cpc, cs:ce],
                        start=True,
                        stop=True,
                        tile_position=(0, 32 * cpc),
                    )
                nc.scalar.copy(xout[:, cs:ce], xop)
            for cpc in range(4):
                # keep the sync queue free for wrep/xt and the scalar queue
                # free for drains: out-write triggers go to GpSimd's SWDGE
                nc.gpsimd.dma_start(
                    out=out_r[o, g, cpc, :, :],
                    in_=xout[32 * cpc:32 * cpc + 16, :],
                )

    with tc.tile_pool(name="wrep", bufs=3) as wrepp:
        wrepp_box["pool"] = wrepp
        with tc.tile_pool(name="frames", bufs=1) as frames, \
             tc.tile_pool(name="stage", bufs=2) as stage, \
             tc.tile_pool(name="prods", bufs=3) as prods:
            frames_pools["pools"] = (frames, stage, prods)
            corr_phase(0)
            conv_phase(0)
            # prefetch phase2(g0)'s inputs so they transfer during ct1
            load_xt(0)
            for o in range(3):
                wrep_tiles[(0, o)] = make_wrep(0, o)
            corr_phase(1)
        with tc.tile_pool(name="pr2", bufs=2) as pr2p, \
             tc.tile_pool(name="xob", bufs=2) as xobp:
            phase2(0, pr2p, xobp)
            conv_phase(1)
            phase2(1, pr2p, xobp)
            phase2(2, pr2p, xobp)
            phase2(3, pr2p, xobp)


def _get_module(loop_k=1):
    key = f"nc{loop_k}"
    if key not in _CACHE:
        _CACHE[key] = _build_module(loop_k)
    return _CACHE[key]


def _consts(conv_w, conv_b):
    conv_w = np.asarray(conv_w, np.float32)
    conv_b = np.asarray(conv_b, np.float32)
    # block-diagonal fused conv weights per group-pair, k-rows permuted to
    # the col-tiled corr layout: corr index i, channel-half gh lives at
    # k-row 32*(i%4) + 2*(i//4) + gh.  Bias (+1.0 residual when t==o)
    # applied at the PSUM drain as a per-partition activation bias (bvec).
    wf2 = np.zeros((128, 2, 128), np.float32)
    bvec = np.zeros((128, 2), np.float32)
    for gp in range(2):
        for gh in range(2):
            g = gp * 2 + gh
            for o in range(TO):
                for t in range(T):
                    m = gh * 64 + o * 8 + t
                    for i in range(63):
                        wf2[_corr_row(i, gh), gp, m] = conv_w[g, o * 8 + t, i]
                    bvec[m, gp] = conv_b[g, o * 8 + t] + (1.0 if t == o else 0.0)

    # shifted ones matrix for the col-tiled channel-sum:
    # lhsT for product r is bm[:, 62-2q : 94-2q] (q = r//4) and must have
    # ones at [0:64, m=2q] and [64:128, m=2q+1]
    bm = np.zeros((128, 96), np.float32)
    bm[0:64, 62] = 1.0
    bm[64:128, 63] = 1.0

    # t-reduce ones per col group: to[p=(cpk,t), cpc, m] = 1 iff m == cpk
    to = np.zeros((128, 4, 16), np.float32)
    for cpc in range(4):
        for cpk in range(16):
            to[cpk * 8:(cpk + 1) * 8, cpc, cpk] = 1.0

    return (
        wf2,
        bm.astype(ml_dtypes.bfloat16),
        to.astype(ml_dtypes.bfloat16),
        bvec,
    )


def kernel(x, conv_w, conv_b):
    from concourse.bass_utils import run_bass_kernel_spmd

    nc = _get_module()
    wf, bm, to, bv = _consts(conv_w, conv_b)
    x = np.asarray(x, np.float32).astype(ml_dtypes.bfloat16)
    x8 = np.ascontiguousarray(x.reshape(NCORES, T, C, H, W))
    in_maps = [
        {
            "xin": np.ascontiguousarray(x8[i]),
            "wf2": wf,
            "bm2": bm,
            "tones": to,
            "bvec": bv,
        }
        for i in range(NCORES)
    ]
    res = run_bass_kernel_spmd(nc, in_maps, core_ids=list(range(NCORES)))
    outs = [r["out"] for r in res.results]
    return np.concatenate(outs, axis=0).astype(np.float32)


# revision 44
# speedup vs baseline: 1.0032x; 1.0032x over previous
"""CorrelationFusion Trainium2 kernel.

Per-clip math (T=8 frames, G=4 groups, 3x3 correlation window):
  corr[g, tt*9+ij, p] = sum_cp x[tt, g*64+cp, p] * xpad[tt+1, g*64+cp, p+d(ij)]
  wx[g, o*8+t, p]     = sum_i conv_w[g, o*8+t, i]*corr[g, i, p] + conv_b[g, o*8+t]
  out[o, g*64+cp, p]  = sum_t wx'[g, o*8+t, p] * x[t, cp*4+g, p]
  (wx' = wx + 1 on the t==o rows -- the residual folded into the conv bias)

Mapping:
  - per-pixel products on VectorE in bf16 (2x mode), channels on partitions
  - partition reductions via TensorE matmuls, 4-way COLUMN-TILED
    (tile_position col groups) so 4 small-M matmuls run concurrently
  - the 1x1 grouped conv is a block-diagonal matmul over the 63 corr rows
    (k-rows permuted to match the col-tiled corr psum layout)
  - padded frame tiles filled directly by strided DMA (no staging copies);
    edge replication pads on ScalarE
  - wx rows replicated into the (cp16, t8)-interleaved layout via a
    broadcast-read DMA from a DRAM bounce buffer
  - data-parallel over the 8 clips: one clip per NeuronCore
"""

import numpy as np
import ml_dtypes

T = 8
TO = 8
G = 4
CPG = 64
C = 256
H = 56
W = 56
PIX = H * W
NCORES = 8
PH = 58   # padded tile rows
PW = 60   # padded tile cols (extra pad for 4B alignment of bf16 rows)
NCH = 7   # pixel chunks per image
CHW = 8   # rows per chunk
CHN = CHW * W  # 448 pixels per chunk
_CACHE = {}


def _corr_row(i, gh):
    """psum/partition row of corr index i (0..62) for channel-half gh."""
    return 32 * (i % 4) + 2 * (i // 4) + gh


def _build_module(loop_k=1):
    import contextlib

    import concourse.bass as bass
    import concourse.bacc as bacc
    import concourse.mybir as mybir
    import concourse.tile as tile

    fp32 = mybir.dt.float32
    bf16 = mybir.dt.bfloat16

    nc = bacc.Bacc(name="corrfusion")
    xin = nc.dram_tensor("xin", [T, C, H, W], bf16, kind="ExternalInput")
    wf2 = nc.dram_tensor("wf2", [128, 2, 128], fp32, kind="ExternalInput")
    bm2 = nc.dram_tensor("bm2", [128, 96], bf16, kind="ExternalInput")
    tones = nc.dram_tensor("tones", [128, 4, 16], bf16, kind="ExternalInput")
    bvec = nc.dram_tensor("bvec", [128, 2], fp32, kind="ExternalInput")
    out = nc.dram_tensor("out", [TO, C, H, W], fp32, kind="ExternalOutput")

    xin_base = xin[:, :, :, :]                                 # base AP for manual APs
    out_r = out.rearrange("o (g cpc k) h w -> o g cpc k (h w)", g=4, cpc=4, k=16)

    with tile.TileContext(nc) as tc:
        with tc.tile_pool(name="consts", bufs=1) as consts, \
             tc.tile_pool(name="corrbuf", bufs=1) as corrbuf, \
             tc.tile_pool(name="xt", bufs=2) as xtp, \
             tc.tile_pool(name="wxdp", bufs=1, space="DRAM") as wxdp, \
             tc.tile_pool(name="psum", bufs=1, space="PSUM") as psum:

            wf_sb = consts.tile([128, 2, 128], fp32)
            nc.scalar.dma_start(out=wf_sb, in_=wf2[:, :, :])
            bm_sb = consts.tile([128, 96], bf16)
            nc.scalar.dma_start(out=bm_sb, in_=bm2[:, :])
            to_sb = consts.tile([128, 4, 16], bf16)
            nc.scalar.dma_start(out=to_sb, in_=tones[:, :, :])
            bv_sb = consts.tile([128, 2], fp32)
            nc.scalar.dma_start(out=bv_sb, in_=bvec[:, :])

            corr_sb = [
                corrbuf.tile([128, PIX], fp32, tag=f"corr{i}", name=f"corr{i}")
                for i in range(2)
            ]
            wx_sb = [
                corrbuf.tile([128, PIX], bf16, tag=f"wx{i}", name=f"wx{i}")
                for i in range(2)
            ]
            loop_cm = (
                tc.For_i(0, loop_k, 1) if loop_k > 1 else contextlib.nullcontext()
            )
            with loop_cm:
                _build_body(nc, tc, tile, bass, mybir, fp32, bf16, xin, xin_base,
                            out_r, wf_sb, bm_sb, to_sb, bv_sb, corr_sb, wx_sb,
                            xtp, wxdp, psum)
    nc.compile()
    return nc


def _build_body(nc, tc, tile, bass, mybir, fp32, bf16, xin, xin_base, out_r,
                wf_sb, bm_sb, to_sb, bv_sb, corr_sb, wx_sb, xtp, wxdp, psum):
    # rows 126/127 (unused by the col-tiled corr layout) must read as zeros
    # in the conv matmul; memset the whole 32-row group (engines need a
    # 32-aligned partition base), the drains overwrite rows 96..125
    for i in range(2):
        nc.vector.memset(corr_sb[i][96:128, :], 0.0)

    wxd = [None, None]

    frames_pools = {}  # set by the caller: frames, stage, prods
    ptiles = {0: {}, 1: {}}
    stiles = {0: {}, 1: {}}

    def load_frame(ct, t, warmup=False):
        frames, stage, prods = frames_pools["pools"]
        ptile, stile = ptiles[ct], stiles[ct]
        P = frames.tile([128, PH, PW], bf16, tag=f"P{t % 4}", name=f"P{ct}_{t}")
        # contiguous DMA into a staging tile; strided center copies on
        # compute engines (strided DMA writes are packet-inefficient)
        stg = stage.tile([128, PIX], bf16, tag="fstage", name="stg")
        nc.sync.dma_start(out=stg, in_=xin[t, ct * 128:(ct + 1) * 128, :, :])
        stg3 = stg.rearrange("p (h w) -> p h w", h=H)

        # during warmup VectorE is idle -- let it carry the S-tile work
        def scopy(o_, i_):
            if warmup:
                nc.vector.tensor_copy(o_, i_)
            else:
                nc.scalar.copy(o_, i_)

        if warmup:
            nc.vector.tensor_copy(P[:, 1:57, 2:58], stg3)
        else:
            nc.scalar.copy(P[:, 1:57, 2:58], stg3)
        ptile[t] = P
        if t > 0:
            # frame 0 is only ever the 'a' operand: its S tile and the
            # P row pads (which serve di-shifted 'b' reads) are never read
            nc.scalar.copy(P[:, 0:1, 2:58], P[:, 1:2, 2:58])
            nc.scalar.copy(P[:, 57:58, 2:58], P[:, 56:57, 2:58])
            S = frames.tile([128, PH, PW], bf16, tag=f"S{t % 4}", name=f"S{ct}_{t}")
            scopy(S[:, 1:57, 1:57], stg3)
            # S is read at cols 0:56 / 2:58 (dj=-1/+1): col pads 0 and 57
            scopy(S[:, 1:57, 0:1], S[:, 1:57, 1:2])
            scopy(S[:, 1:57, 57:58], S[:, 1:57, 56:57])
            scopy(S[:, 0:1, 0:58], S[:, 1:2, 0:58])
            scopy(S[:, 57:58, 0:58], S[:, 56:57, 0:58])
            stile[t] = S

    def corr_phase(ct):
        if True:
            frames, stage, prods = frames_pools["pools"]
            cps = [
                psum.tile([128, CHN], fp32, tag=f"b{c}", name=f"cps{ct}_{c}")
                for c in range(NCH)
            ]
            ptile = ptiles[ct]
            stile = stiles[ct]

            if 0 not in ptile:
                load_frame(ct, 0, warmup=(ct == 0))
            if 1 not in ptile:
                load_frame(ct, 1, warmup=(ct == 0))
            for tt in range(T - 1):
                if tt + 2 < T:
                    load_frame(ct, tt + 2)
                if ct == 0 and tt == T - 2:
                    # preload next half's first frames while this half's
                    # tail products still run (keeps the scalar queue from
                    # serializing drains ahead of them)
                    load_frame(1, 0)
                    load_frame(1, 1)
                a = ptile[tt][:, 1:57, 2:58]
                for ij in range(9):
                    di = ij // 3 - 1
                    dj = ij % 3 - 1
                    r = tt * 9 + ij
                    if dj == 0:
                        b = ptile[tt + 1][:, 1 + di:57 + di, 2:58]
                    elif dj == 1:
                        b = stile[tt + 1][:, 1 + di:57 + di, 2:58]
                    else:
                        b = stile[tt + 1][:, 1 + di:57 + di, 0:56]
                    pr = prods.tile([128, PIX], bf16, tag="prod", name="pr")
                    pr3 = pr.rearrange("p (h w) -> p h w", h=H)
                    nc.vector.tensor_mul(pr3, a, b)
                    # col-tiled reduction: product r -> col group r%4,
                    # rows 2*(r//4)+gh; lhsT = 32-col slice of the shifted
                    # ones matrix
                    cg = r % 4
                    q = r // 4
                    lhsT = bm_sb[:, 62 - 2 * q:94 - 2 * q]
                    last = (q == 15) or (cg == 3 and q == 14)
                    for c in range(NCH):
                        nc.tensor.matmul(
                            cps[c][32 * cg:32 * cg + 32, :],
                            lhsT,
                            pr[:, c * CHN:(c + 1) * CHN],
                            start=(q == 0),
                            stop=last,
                            tile_position=(0, 32 * cg),
                        )
            for c in range(NCH):
                nc.scalar.copy(
                    corr_sb[ct][0:126, c * CHN:(c + 1) * CHN],
                    cps[c][0:126, :],
                )

    def conv_phase(gp):
        # grouped 1x1 conv (+bias +residual); bounce wx to DRAM so the
        # per-(o,g) replication is one broadcast-read DMA
        for c in range(NCH):
            wpp = psum.tile([128, CHN], fp32, tag="wp", name=f"wpp{gp}_{c}")
            nc.tensor.matmul(
                wpp,
                wf_sb[:, gp, :],
                corr_sb[gp][:, c * CHN:(c + 1) * CHN],
                start=True,
                stop=True,
            )
            nc.scalar.activation(
                wx_sb[gp][:, c * CHN:(c + 1) * CHN],
                wpp,
                mybir.ActivationFunctionType.Identity,
                bias=bv_sb[:, gp:gp + 1],
                scale=1.0,
            )
        # trigger on the scalar queue: its dep (the wx activations) sits
        # right before it there, so it never blocks a queue head
        wd = wxdp.tile([128, PIX], bf16, tag=f"wxd{gp}", name=f"wxd{gp}")
        nc.scalar.dma_start(out=wd, in_=wx_sb[gp])
        wxd[gp] = wd

    xt_tiles = {}
    wrep_tiles = {}
    wrepp_box = {}

    def load_xt(g):
        if g in xt_tiles:
            return xt_tiles[g]
        xt = xtp.tile([128, 4, PIX], bf16, tag="xt", name=f"xt{g}")
        for cpc in range(4):
            # partition = (cpk, t): channel c = cpc*64 + cpk*4 + g
            src = bass.AP(
                tensor=xin_base.tensor,
                offset=(cpc * 64 + g) * PIX,
                ap=[[4 * PIX, 16], [C * PIX, T], [1, PIX]],
            )
            nc.sync.dma_start(out=xt[:, cpc, :], in_=src)
        xt_tiles[g] = xt
        return xt

    def make_wrep(g, o):
        if (g, o) in wrep_tiles:
            return wrep_tiles.pop((g, o))
        rowbase = (g % 2) * 64 + o * 8
        wrep = wrepp_box["pool"].tile([128, PIX], bf16, tag="wrep", name="wrep")
        wsrc = bass.AP(
            tensor=wxd[g // 2].tensor,
            offset=wxd[g // 2].offset + rowbase * PIX,
            ap=[[0, 16], [PIX, 8], [1, PIX]],
        )
        nc.sync.dma_start(out=wrep, in_=wsrc)
        return wrep

    def phase2(g, pr2p, xobp):
        xt = load_xt(g)
        if g + 1 < G:
            # prefetch the next group's inputs (its wxd half exists by now)
            load_xt(g + 1)
            wrep_tiles[(g + 1, 0)] = make_wrep(g + 1, 0)
        for o in range(TO):
            wrep = make_wrep(g, o)
            # for the very last (g, o) compute products chunk-wise so the
            # trailing MM/drain/DMA chain starts ~6us earlier
            fine = (g == 3 and o == TO - 1)
            # cpc-pair products in one DVE op: wrep broadcast over cpc
            pr4 = pr2p.tile([128, 4, PIX], bf16, tag="pr2", bufs=2, name="pr4")
            if not fine:
                wb = wrep[:, :].unsqueeze(1).broadcast_to((128, 2, PIX))
                nc.vector.tensor_mul(pr4[:, 0:2, :], xt[:, 0:2, :], wb)
                nc.vector.tensor_mul(pr4[:, 2:4, :], xt[:, 2:4, :], wb)
            xout = xobp.tile([128, PIX], fp32, tag="xout", name="xout")
            for c in range(NCH):
                cs, ce = c * CHN, (c + 1) * CHN
                if fine:
                    wbc = wrep[:, cs:ce].unsqueeze(1).broadcast_to((128, 4, CHN))
                    nc.vector.tensor_mul(
                        pr4[:, :, cs:ce], xt[:, :, cs:ce], wbc
                    )
                xop = psum.tile([128, CHN], fp32, tag=f"b{c}", name=f"xo{g}_{o}_{c}")
                # 4 col-tiled t-reductions run concurrently
                for cpc in range(4):
                    nc.tensor.matmul(
                        xop[32 * cpc:32 * cpc + 16, :],
                        to_sb[:, cpc, :],
                        pr4[:, cpc, cs:ce],
                        start=True,
                        stop=True,
                        tile_position=(0, 32 * cpc),
                    )
                nc.scalar.copy(xout[:, cs:ce], xop)
            for cpc in range(4):
                # keep the sync queue free for wrep/xt and the scalar queue
                # free for drains: out-write triggers go to GpSimd's SWDGE
                nc.gpsimd.dma_start(
                    out=out_r[o, g, cpc, :, :],
                    in_=xout[32 * cpc:32 * cpc + 16, :],
                )

    with tc.tile_pool(name="wrep", bufs=4) as wrepp:
        wrepp_box["pool"] = wrepp
        with tc.tile_pool(name="frames", bufs=1) as frames, \
             tc.tile_pool(name="stage", bufs=2) as stage, \
             tc.tile_pool(name="prods", bufs=3) as prods:
            frames_pools["pools"] = (frames, stage, prods)
            corr_phase(0)
            conv_phase(0)
            # prefetch phase2(g0)'s inputs so they transfer during ct1
            load_xt(0)
            for o in range(3):
                wrep_tiles[(0, o)] = make_wrep(0, o)
            corr_phase(1)
        with tc.tile_pool(name="pr2", bufs=2) as pr2p, \
             tc.tile_pool(name="xob", bufs=3) as xobp:
            phase2(0, pr2p, xobp)
            conv_phase(1)
            phase2(1, pr2p, xobp)
            phase2(2, pr2p, xobp)
            phase2(3, pr2p, xobp)


def _get_module(loop_k=1):
    key = f"nc{loop_k}"
    if key not in _CACHE:
        _CACHE[key] = _build_module(loop_k)
    return _CACHE[key]


def _consts(conv_w, conv_b):
    conv_w = np.asarray(conv_w, np.float32)
    conv_b = np.asarray(conv_b, np.float32)
    # block-diagonal fused conv weights per group-pair, k-rows permuted to
    # the col-tiled corr layout: corr index i, channel-half gh lives at
    # k-row 32*(i%4) + 2*(i//4) + gh.  Bias (+1.0 residual when t==o)
    # applied at the PSUM drain as a per-partition activation bias (bvec).
    wf2 = np.zeros((128, 2, 128), np.float32)
    bvec = np.zeros((128, 2), np.float32)
    for gp in range(2):
        for gh in range(2):
            g = gp * 2 + gh
            for o in range(TO):
                for t in range(T):
                    m = gh * 64 + o * 8 + t
                    for i in range(63):
                        wf2[_corr_row(i, gh), gp, m] = conv_w[g, o * 8 + t, i]
                    bvec[m, gp] = conv_b[g, o * 8 + t] + (1.0 if t == o else 0.0)

    # shifted ones matrix for the col-tiled channel-sum:
    # lhsT for product r is bm[:, 62-2q : 94-2q] (q = r//4) and must have
    # ones at [0:64, m=2q] and [64:128, m=2q+1]
    bm = np.zeros((128, 96), np.float32)
    bm[0:64, 62] = 1.0
    bm[64:128, 63] = 1.0

    # t-reduce ones per col group: to[p=(cpk,t), cpc, m] = 1 iff m == cpk
    to = np.zeros((128, 4, 16), np.float32)
    for cpc in range(4):
        for cpk in range(16):
            to[cpk * 8:(cpk + 1) * 8, cpc, cpk] = 1.0

    return (
        wf2,
        bm.astype(ml_dtypes.bfloat16),
        to.astype(ml_dtypes.bfloat16),
        bvec,
    )


def kernel(x, conv_w, conv_b):
    from concourse.bass_utils import run_bass_kernel_spmd

    nc = _get_module()
    wf, bm, to, bv = _consts(conv_w, conv_b)
    x = np.asarray(x, np.float32).astype(ml_dtypes.bfloat16)
    x8 = np.ascontiguousarray(x.reshape(NCORES, T, C, H, W))
    in_maps = [
        {
            "xin": np.ascontiguousarray(x8[i]),
            "wf2": wf,
            "bm2": bm,
            "tones": to,
            "bvec": bv,
        }
        for i in range(NCORES)
    ]
    res = run_bass_kernel_spmd(nc, in_maps, core_ids=list(range(NCORES)))
    outs = [r["out"] for r in res.results]
    return np.concatenate(outs, axis=0).astype(np.float32)


# revision 45
# speedup vs baseline: 1.0408x; 1.0374x over previous
"""CorrelationFusion Trainium2 kernel.

Per-clip math (T=8 frames, G=4 groups, 3x3 correlation window):
  corr[g, tt*9+ij, p] = sum_cp x[tt, g*64+cp, p] * xpad[tt+1, g*64+cp, p+d(ij)]
  wx[g, o*8+t, p]     = sum_i conv_w[g, o*8+t, i]*corr[g, i, p] + conv_b[g, o*8+t]
  out[o, g*64+cp, p]  = sum_t wx'[g, o*8+t, p] * x[t, cp*4+g, p]
  (wx' = wx + 1 on the t==o rows -- the residual folded into the conv bias)

Mapping:
  - per-pixel products on VectorE in bf16 (2x mode), channels on partitions
  - partition reductions via TensorE matmuls, 4-way COLUMN-TILED
    (tile_position col groups) so 4 small-M matmuls run concurrently
  - the 1x1 grouped conv is a block-diagonal matmul over the 63 corr rows
    (k-rows permuted to match the col-tiled corr psum layout)
  - padded frame tiles filled directly by strided DMA (no staging copies);
    edge replication pads on ScalarE
  - wx rows replicated into the (cp16, t8)-interleaved layout via a
    broadcast-read DMA from a DRAM bounce buffer
  - data-parallel over the 8 clips: one clip per NeuronCore
"""

import numpy as np
import ml_dtypes

T = 8
TO = 8
G = 4
CPG = 64
C = 256
H = 56
W = 56
PIX = H * W
NCORES = 8
PH = 58   # padded tile rows
PW = 60   # padded tile cols (extra pad for 4B alignment of bf16 rows)
NCH = 7   # pixel chunks per image
CHW = 8   # rows per chunk
CHN = CHW * W  # 448 pixels per chunk
_CACHE = {}


def _corr_row(i, gh):
    """psum/partition row of corr index i (0..62) for channel-half gh."""
    return 32 * (i % 4) + 2 * (i // 4) + gh


def _build_module(loop_k=1):
    import contextlib

    import concourse.bass as bass
    import concourse.bacc as bacc
    import concourse.mybir as mybir
    import concourse.tile as tile

    fp32 = mybir.dt.float32
    bf16 = mybir.dt.bfloat16

    nc = bacc.Bacc(name="corrfusion")
    xin = nc.dram_tensor("xin", [T, C, H, W], bf16, kind="ExternalInput")
    wf2 = nc.dram_tensor("wf2", [128, 2, 128], fp32, kind="ExternalInput")
    bm2 = nc.dram_tensor("bm2", [128, 96], bf16, kind="ExternalInput")
    tones = nc.dram_tensor("tones", [128, 4, 16], bf16, kind="ExternalInput")
    bvec = nc.dram_tensor("bvec", [128, 2], fp32, kind="ExternalInput")
    out = nc.dram_tensor("out", [TO, C, H, W], fp32, kind="ExternalOutput")

    xin_base = xin[:, :, :, :]                                 # base AP for manual APs
    out_r = out.rearrange("o (g cpc k) h w -> o g cpc k (h w)", g=4, cpc=4, k=16)

    with tile.TileContext(nc) as tc:
        with tc.tile_pool(name="consts", bufs=1) as consts, \
             tc.tile_pool(name="corrbuf", bufs=1) as corrbuf, \
             tc.tile_pool(name="xt", bufs=2) as xtp, \
             tc.tile_pool(name="wxdp", bufs=1, space="DRAM") as wxdp, \
             tc.tile_pool(name="psum", bufs=1, space="PSUM") as psum:

            wf_sb = consts.tile([128, 2, 128], fp32)
            nc.scalar.dma_start(out=wf_sb, in_=wf2[:, :, :])
            bm_sb = consts.tile([128, 96], bf16)
            nc.scalar.dma_start(out=bm_sb, in_=bm2[:, :])
            to_sb = consts.tile([128, 4, 16], bf16)
            nc.scalar.dma_start(out=to_sb, in_=tones[:, :, :])
            bv_sb = consts.tile([128, 2], fp32)
            nc.scalar.dma_start(out=bv_sb, in_=bvec[:, :])

            corr_sb = [
                corrbuf.tile([128, PIX], fp32, tag=f"corr{i}", name=f"corr{i}")
                for i in range(2)
            ]
            wx_sb = [
                corrbuf.tile([128, PIX], bf16, tag=f"wx{i}", name=f"wx{i}")
                for i in range(2)
            ]
            loop_cm = (
                tc.For_i(0, loop_k, 1) if loop_k > 1 else contextlib.nullcontext()
            )
            with loop_cm:
                _build_body(nc, tc, tile, bass, mybir, fp32, bf16, xin, xin_base,
                            out_r, wf_sb, bm_sb, to_sb, bv_sb, corr_sb, wx_sb,
                            xtp, wxdp, psum)
    nc.compile()
    return nc


def _build_body(nc, tc, tile, bass, mybir, fp32, bf16, xin, xin_base, out_r,
                wf_sb, bm_sb, to_sb, bv_sb, corr_sb, wx_sb, xtp, wxdp, psum):
    # rows 126/127 (unused by the col-tiled corr layout) must read as zeros
    # in the conv matmul; memset the whole 32-row group (engines need a
    # 32-aligned partition base), the drains overwrite rows 96..125
    for i in range(2):
        nc.vector.memset(corr_sb[i][96:128, :], 0.0)

    wxd = [None, None]

    frames_pools = {}  # set by the caller: frames, stage, prods
    ptiles = {0: {}, 1: {}}
    stiles = {0: {}, 1: {}}

    def load_frame(ct, t, warmup=False):
        frames, stage, prods = frames_pools["pools"]
        ptile, stile = ptiles[ct], stiles[ct]
        P = frames.tile([128, PH, PW], bf16, tag=f"P{t % 4}", name=f"P{ct}_{t}")
        # contiguous DMA into a staging tile; strided center copies on
        # compute engines (strided DMA writes are packet-inefficient)
        stg = stage.tile([128, PIX], bf16, tag="fstage", name="stg")
        nc.sync.dma_start(out=stg, in_=xin[t, ct * 128:(ct + 1) * 128, :, :])
        stg3 = stg.rearrange("p (h w) -> p h w", h=H)

        # during warmup VectorE is idle -- let it carry the S-tile work
        def scopy(o_, i_):
            if warmup:
                nc.vector.tensor_copy(o_, i_)
            else:
                nc.scalar.copy(o_, i_)

        if warmup:
            nc.vector.tensor_copy(P[:, 1:57, 2:58], stg3)
        else:
            nc.scalar.copy(P[:, 1:57, 2:58], stg3)
        ptile[t] = P
        if t > 0:
            # frame 0 is only ever the 'a' operand: its S tile and the
            # P row pads (which serve di-shifted 'b' reads) are never read
            nc.scalar.copy(P[:, 0:1, 2:58], P[:, 1:2, 2:58])
            nc.scalar.copy(P[:, 57:58, 2:58], P[:, 56:57, 2:58])
            S = frames.tile([128, PH, PW], bf16, tag=f"S{t % 4}", name=f"S{ct}_{t}")
            scopy(S[:, 1:57, 1:57], stg3)
            # S is read at cols 0:56 / 2:58 (dj=-1/+1): col pads 0 and 57
            scopy(S[:, 1:57, 0:1], S[:, 1:57, 1:2])
            scopy(S[:, 1:57, 57:58], S[:, 1:57, 56:57])
            scopy(S[:, 0:1, 0:58], S[:, 1:2, 0:58])
            scopy(S[:, 57:58, 0:58], S[:, 56:57, 0:58])
            stile[t] = S

    def corr_phase(ct):
        if True:
            frames, stage, prods = frames_pools["pools"]
            cps = [
                psum.tile([128, CHN], fp32, tag=f"b{c}", name=f"cps{ct}_{c}")
                for c in range(NCH)
            ]
            ptile = ptiles[ct]
            stile = stiles[ct]

            if 0 not in ptile:
                load_frame(ct, 0, warmup=(ct == 0))
            if 1 not in ptile:
                load_frame(ct, 1, warmup=(ct == 0))
            for tt in range(T - 1):
                if tt + 2 < T:
                    load_frame(ct, tt + 2)
                if ct == 0 and tt == T - 2:
                    # preload next half's first frames while this half's
                    # tail products still run (keeps the scalar queue from
                    # serializing drains ahead of them)
                    load_frame(1, 0)
                    load_frame(1, 1)
                a = ptile[tt][:, 1:57, 2:58]
                for ij in range(9):
                    di = ij // 3 - 1
                    dj = ij % 3 - 1
                    r = tt * 9 + ij
                    if dj == 0:
                        b = ptile[tt + 1][:, 1 + di:57 + di, 2:58]
                    elif dj == 1:
                        b = stile[tt + 1][:, 1 + di:57 + di, 2:58]
                    else:
                        b = stile[tt + 1][:, 1 + di:57 + di, 0:56]
                    pr = prods.tile([128, PIX], bf16, tag="prod", name="pr")
                    pr3 = pr.rearrange("p (h w) -> p h w", h=H)
                    nc.vector.tensor_mul(pr3, a, b)
                    # col-tiled reduction: product r -> col group r%4,
                    # rows 2*(r//4)+gh; lhsT = 32-col slice of the shifted
                    # ones matrix
                    cg = r % 4
                    q = r // 4
                    lhsT = bm_sb[:, 62 - 2 * q:94 - 2 * q]
                    last = (q == 15) or (cg == 3 and q == 14)
                    for c in range(NCH):
                        nc.tensor.matmul(
                            cps[c][32 * cg:32 * cg + 32, :],
                            lhsT,
                            pr[:, c * CHN:(c + 1) * CHN],
                            start=(q == 0),
                            stop=last,
                            tile_position=(0, 32 * cg),
                        )
            for c in range(NCH):
                nc.scalar.copy(
                    corr_sb[ct][0:126, c * CHN:(c + 1) * CHN],
                    cps[c][0:126, :],
                )

    def conv_phase(gp):
        # grouped 1x1 conv (+bias +residual); bounce wx to DRAM so the
        # per-(o,g) replication is one broadcast-read DMA
        for c in range(NCH):
            wpp = psum.tile([128, CHN], fp32, tag="wp", name=f"wpp{gp}_{c}")
            nc.tensor.matmul(
                wpp,
                wf_sb[:, gp, :],
                corr_sb[gp][:, c * CHN:(c + 1) * CHN],
                start=True,
                stop=True,
            )
            nc.scalar.activation(
                wx_sb[gp][:, c * CHN:(c + 1) * CHN],
                wpp,
                mybir.ActivationFunctionType.Identity,
                bias=bv_sb[:, gp:gp + 1],
                scale=1.0,
            )
        # trigger on the scalar queue: its dep (the wx activations) sits
        # right before it there, so it never blocks a queue head
        wd = wxdp.tile([128, PIX], bf16, tag=f"wxd{gp}", name=f"wxd{gp}")
        nc.scalar.dma_start(out=wd, in_=wx_sb[gp])
        wxd[gp] = wd

    xt_tiles = {}
    wrep_tiles = {}
    wrepp_box = {}

    def load_xt(g):
        if g in xt_tiles:
            return xt_tiles[g]
        xt = xtp.tile([128, 4, PIX], bf16, tag="xt", name=f"xt{g}")
        for cpc in range(4):
            # partition = (cpk, t): channel c = cpc*64 + cpk*4 + g
            src = bass.AP(
                tensor=xin_base.tensor,
                offset=(cpc * 64 + g) * PIX,
                ap=[[4 * PIX, 16], [C * PIX, T], [1, PIX]],
            )
            nc.sync.dma_start(out=xt[:, cpc, :], in_=src)
        xt_tiles[g] = xt
        return xt

    def make_wrep(g, o):
        if (g, o) in wrep_tiles:
            return wrep_tiles.pop((g, o))
        rowbase = (g % 2) * 64 + o * 8
        wrep = wrepp_box["pool"].tile([128, PIX], bf16, tag="wrep", name="wrep")
        wsrc = bass.AP(
            tensor=wxd[g // 2].tensor,
            offset=wxd[g // 2].offset + rowbase * PIX,
            ap=[[0, 16], [PIX, 8], [1, PIX]],
        )
        nc.sync.dma_start(out=wrep, in_=wsrc)
        return wrep

    def phase2(g, pr2p, xobp):
        xt = load_xt(g)
        for o in range(TO):
            wrep = make_wrep(g, o)
            # for the very last (g, o) compute products chunk-wise so the
            # trailing MM/drain/DMA chain starts ~6us earlier
            fine = (g == 3 and o == TO - 1)
            # cpc-pair products in one DVE op: wrep broadcast over cpc
            pr4 = pr2p.tile([128, 4, PIX], bf16, tag="pr2", bufs=2, name="pr4")
            if not fine:
                wb = wrep[:, :].unsqueeze(1).broadcast_to((128, 2, PIX))
                nc.vector.tensor_mul(pr4[:, 0:2, :], xt[:, 0:2, :], wb)
                nc.vector.tensor_mul(pr4[:, 2:4, :], xt[:, 2:4, :], wb)
            xout = xobp.tile([128, PIX], fp32, tag="xout", name="xout")
            for c in range(NCH):
                cs, ce = c * CHN, (c + 1) * CHN
                if fine:
                    wbc = wrep[:, cs:ce].unsqueeze(1).broadcast_to((128, 4, CHN))
                    nc.vector.tensor_mul(
                        pr4[:, :, cs:ce], xt[:, :, cs:ce], wbc
                    )
                xop = psum.tile([128, CHN], fp32, tag=f"b{c}", name=f"xo{g}_{o}_{c}")
                # 4 col-tiled t-reductions run concurrently
                for cpc in range(4):
                    nc.tensor.matmul(
                        xop[32 * cpc:32 * cpc + 16, :],
                        to_sb[:, cpc, :],
                        pr4[:, cpc, cs:ce],
                        start=True,
                        stop=True,
                        tile_position=(0, 32 * cpc),
                    )
                nc.scalar.copy(xout[:, cs:ce], xop)
            for cpc in range(4):
                # keep the sync queue free for wrep/xt and the scalar queue
                # free for drains: out-write triggers go to GpSimd's SWDGE
                nc.gpsimd.dma_start(
                    out=out_r[o, g, cpc, :, :],
                    in_=xout[32 * cpc:32 * cpc + 16, :],
                )

    with tc.tile_pool(name="wrep", bufs=4) as wrepp:
        wrepp_box["pool"] = wrepp
        with tc.tile_pool(name="frames", bufs=1) as frames, \
             tc.tile_pool(name="stage", bufs=2) as stage, \
             tc.tile_pool(name="prods", bufs=3) as prods:
            frames_pools["pools"] = (frames, stage, prods)
            corr_phase(0)
            conv_phase(0)
            # prefetch phase2(g0)'s inputs so they transfer during ct1
            load_xt(0)
            for o in range(3):
                wrep_tiles[(0, o)] = make_wrep(0, o)
            corr_phase(1)
        with tc.tile_pool(name="pr2", bufs=2) as pr2p, \
             tc.tile_pool(name="xob", bufs=3) as xobp:
            phase2(0, pr2p, xobp)
            conv_phase(1)
            phase2(1, pr2p, xobp)
            phase2(2, pr2p, xobp)
            phase2(3, pr2p, xobp)


def _get_module(loop_k=1):
    key = f"nc{loop_k}"
    if key not in _CACHE:
        _CACHE[key] = _build_module(loop_k)
    return _CACHE[key]


def _consts(conv_w, conv_b):
    conv_w = np.asarray(conv_w, np.float32)
    conv_b = np.asarray(conv_b, np.float32)
    # block-diagonal fused conv weights per group-pair, k-rows permuted to
    # the col-tiled corr layout: corr index i, channel-half gh lives at
    # k-row 32*(i%4) + 2*(i//4) + gh.  Bias (+1.0 residual when t==o)
    # applied at the PSUM drain as a per-partition activation bias (bvec).
    wf2 = np.zeros((128, 2, 128), np.float32)
    bvec = np.zeros((128, 2), np.float32)
    for gp in range(2):
        for gh in range(2):
            g = gp * 2 + gh
            for o in range(TO):
                for t in range(T):
                    m = gh * 64 + o * 8 + t
                    for i in range(63):
                        wf2[_corr_row(i, gh), gp, m] = conv_w[g, o * 8 + t, i]
                    bvec[m, gp] = conv_b[g, o * 8 + t] + (1.0 if t == o else 0.0)

    # shifted ones matrix for the col-tiled channel-sum:
    # lhsT for product r is bm[:, 62-2q : 94-2q] (q = r//4) and must have
    # ones at [0:64, m=2q] and [64:128, m=2q+1]
    bm = np.zeros((128, 96), np.float32)
    bm[0:64, 62] = 1.0
    bm[64:128, 63] = 1.0

    # t-reduce ones per col group: to[p=(cpk,t), cpc, m] = 1 iff m == cpk
    to = np.zeros((128, 4, 16), np.float32)
    for cpc in range(4):
        for cpk in range(16):
            to[cpk * 8:(cpk + 1) * 8, cpc, cpk] = 1.0

    return (
        wf2,
        bm.astype(ml_dtypes.bfloat16),
        to.astype(ml_dtypes.bfloat16),
        bvec,
    )


def kernel(x, conv_w, conv_b):
    from concourse.bass_utils import run_bass_kernel_spmd

    nc = _get_module()
    wf, bm, to, bv = _consts(conv_w, conv_b)
    x = np.asarray(x, np.float32).astype(ml_dtypes.bfloat16)
    x8 = np.ascontiguousarray(x.reshape(NCORES, T, C, H, W))
    in_maps = [
        {
            "xin": np.ascontiguousarray(x8[i]),
            "wf2": wf,
            "bm2": bm,
            "tones": to,
            "bvec": bv,
        }
        for i in range(NCORES)
    ]
    res = run_bass_kernel_spmd(nc, in_maps, core_ids=list(range(NCORES)))
    outs = [r["out"] for r in res.results]
    return np.concatenate(outs, axis=0).astype(np.float32)
